# revision 1
# baseline (speedup 1.0000x reference)
"""GATv2 (3-layer, 4-head) message-passing kernel for Trainium2, 8-core SPMD.

Strategy (per sharding hint): nodes sharded contiguously across 8 cores;
edges partitioned by destination; per-layer AllGather of the source-side
transform xl = x @ Wl so each core can gather arbitrary source rows; the
segment softmax / scatter-add stay local per destination shard.

Key algebraic trick: with m_e = xl[src_e] + xr[dst_e] (xr holds x@Wr plus all
per-edge biases) and ee = exp(logit), the weighted sum over incoming edges is
    sum_e alpha_e * xl[src_e] = (sum_e ee_e * m_e)/denom - xr[dst]
so the gathered xl rows never need to be kept separately: the two row-gathers
are fused by a compute-DMA (gather xr rows, then gather-ADD xl rows) and the
per-edge tensor m is consumed directly.

Segment reductions use a 0/1 selection matrix S[e, dst_local] built on-chip by
an is_equal compare against an iota row; one PE matmul per 128-edge k-tile
accumulates both the weighted feature sum and the softmax denominator
(rhs = [z | ee], 132 columns) into PSUM.
"""

import sys

sys.path.insert(0, "/opt/trn_rl_repo")

import ml_dtypes
import numpy as np

import concourse.bass as bass
import concourse.bacc as bacc
import concourse.tile as tile
from concourse import mybir
from concourse.bass import IndirectOffsetOnAxis

F32 = mybir.dt.float32
I32 = mybir.dt.int32
I16 = mybir.dt.int16
AF = mybir.ActivationFunctionType
ALU = mybir.AluOpType
AX = mybir.AxisListType

import os

P = 128
NEG_SLOPE = 0.2
LN_EPS = 1e-5
DENOM_EPS = 1e-30
USE_ACT_LRELU = False  # sim lacks Lrelu; flip to try single-op ACT path on HW
# Per-k-tile [P,1]-offset indirect gathers are the HW-verified path.
# (Multi-column indirect offsets return garbage on HW; InstDMAGatherAnt hangs
# on this runtime.)
SPLIT_GATHER = bool(int(os.environ.get("GAT_SPLIT_GATHER", "1")))
NO_CCE = bool(int(os.environ.get("GAT_NO_CCE", "0")))
DBG_LAYERS = int(os.environ.get("GAT_LAYERS", "0"))  # 0 = all
# V2: xr[dst] per-edge rows come from a PE segment-broadcast (S.T @ xr_chunk)
# instead of indirect gathers (halves the serialized gather count), with bf16
# edge-phase tensors so the extra matmuls ride the FWL fast path.
V2 = bool(int(os.environ.get("GAT_V2", "1")))  # HW-verified: 5.64ms, rel 3.2e-3
# V3: m = psum(ST@xr) + g2 via one DVE add (drops identity-matmul + ACT copy
# from the m-build chain) and deeper edge buffering to keep gathers streaming.
V3 = bool(int(os.environ.get("GAT_V3", "1")))  # HW-verified: rel err 3.2e-3
BF16 = mybir.dt.bfloat16


class Cfg:
    def __init__(self, N=50000, D=128, H=4, L=3, n_cores=8):
        self.N, self.D, self.H, self.L, self.M = N, D, H, L, n_cores
        self.C = D // H
        assert N % n_cores == 0
        self.shard = N // n_cores
        self.chunks = (self.shard + P - 1) // P


# ----------------------------------------------------------------------------
# Host preprocessing: append self loops, sort by dst, pack per-core chunk/k-tile
# arrays.
# ----------------------------------------------------------------------------

def _wrap16(a):
    """Linear idx list [n] -> dma_gather layout [128, n/16] int16 (value for
    gathered row i sits at partition i%16, col i//16; replicated across the 8
    Q7 groups)."""
    n = len(a)
    assert n % 16 == 0
    a16 = a.reshape(-1, 16).T.astype(np.int16)  # [16, n/16]
    return np.ascontiguousarray(np.tile(a16, (8, 1)))


def preprocess(edge_index, cfg):
    N, M, shard, chunks = cfg.N, cfg.M, cfg.shard, cfg.chunks
    half = N // 2
    assert half < 32768 and N - half < 32768 and shard < 32768
    ei = np.asarray(edge_index)
    loops = np.arange(N, dtype=np.int64)
    src = np.concatenate([ei[0].astype(np.int64), loops])
    dst = np.concatenate([ei[1].astype(np.int64), loops])
    order = np.argsort(dst, kind="stable")
    src_s, dst_s = src[order], dst[order]

    # per-(core, chunk) edge lists. In split-gather mode edges pack compactly;
    # in dma_gather mode they are segregated by src half (slot order within a
    # chunk is free — S handles the dst mapping per slot).
    per_core = []
    max_lo = max_hi = 1
    for c in range(M):
        lo, hi = np.searchsorted(dst_s, [c * shard, (c + 1) * shard])
        d_loc = dst_s[lo:hi] - c * shard
        s_loc = src_s[lo:hi]
        ch = d_loc // P
        chunk_edges = []
        for t in range(chunks):
            m = ch == t
            sl, dl = s_loc[m], d_loc[m]
            if SPLIT_GATHER:
                is_lo = np.ones(len(sl), dtype=bool)
            else:
                is_lo = sl < half
            chunk_edges.append(
                (sl[is_lo], dl[is_lo], sl[~is_lo], dl[~is_lo])
            )
            max_lo = max(max_lo, -(-len(sl[is_lo]) // P))
            max_hi = max(max_hi, -(-int((~is_lo).sum()) // P))
        per_core.append(chunk_edges)

    if SPLIT_GATHER:
        KLO, KHI = max_lo, 0
    else:
        KLO, KHI = max_lo, max_hi
    K = KLO + KHI
    meta = {"K": K, "KLO": KLO, "KHI": KHI, "HALF": half}

    pre = []
    for c in range(M):
        src_idx = np.zeros((chunks, P, K), dtype=np.int32)
        dstg = np.zeros((chunks, P, K), dtype=np.int32)
        dstl = np.full((chunks, P, K), 300.0, dtype=np.float32)
        idx_lo = np.zeros((chunks, P, KLO * 8), dtype=np.int16)
        idx_hi = np.zeros((chunks, P, KHI * 8), dtype=np.int16)
        idx_dst = np.zeros((chunks, P, K * 8), dtype=np.int16)
        dstl_kt = np.full((chunks, 1, K * P), 300.0, dtype=np.float32)
        for t in range(chunks):
            sl_lo, dl_lo, sl_hi, dl_hi = per_core[c][t]
            lin_lo = np.zeros(KLO * P, dtype=np.int64)
            lin_lo[: len(sl_lo)] = sl_lo
            lin_hi = np.zeros(KHI * P, dtype=np.int64)
            lin_hi[: len(sl_hi)] = sl_hi - half
            lin_d = np.zeros(K * P, dtype=np.int64)
            lin_d[: len(dl_lo)] = dl_lo
            lin_d[KLO * P : KLO * P + len(dl_hi)] = dl_hi
            idx_lo[t] = _wrap16(lin_lo)
            idx_hi[t] = _wrap16(lin_hi)
            idx_dst[t] = _wrap16(lin_d)
            dstl_kt[t, 0, : len(dl_lo)] = dl_lo - t * P
            dstl_kt[t, 0, KLO * P : KLO * P + len(dl_hi)] = dl_hi - t * P
            # slot (p, k) views for the fp32/int32 per-slot arrays
            for (sarr, darr, k0) in ((sl_lo, dl_lo, 0), (sl_hi, dl_hi, KLO)):
                j = np.arange(len(sarr))
                p, k = j % P, k0 + j // P
                src_idx[t, p, k] = sarr
                dstg[t, p, k] = darr
                dstl[t, p, k] = (darr - t * P).astype(np.float32)
        pre.append(
            {
                "src_idx": src_idx,
                "dstg": dstg,
                "dstl": dstl,
                "idx_lo": idx_lo,
                "idx_hi": idx_hi,
                "idx_dst": idx_dst,
                "dstl16": dstl.astype(ml_dtypes.bfloat16),
                "dstl_kt": dstl_kt.astype(ml_dtypes.bfloat16),
            }
        )
    return pre, meta


# ----------------------------------------------------------------------------
# Kernel builder. io maps logical names to DRAM APs (ExternalInput/Output).
# ----------------------------------------------------------------------------

def build(tc, io, cfg, meta):
    from contextlib import ExitStack

    nc = tc.nc
    D, H, L, C = cfg.D, cfg.H, cfg.L, cfg.C
    K, KLO, KHI, HALF = meta["K"], meta["KLO"], meta["KHI"], meta["HALF"]
    shard, chunks = cfg.shard, cfg.chunks

    ctx = ExitStack()
    dram = ctx.enter_context(tc.tile_pool(name="drampool", bufs=1, space="DRAM"))
    consts = ctx.enter_context(tc.tile_pool(name="consts", bufs=1))
    lconsts = ctx.enter_context(tc.tile_pool(name="lconsts", bufs=2))
    nodep = ctx.enter_context(tc.tile_pool(name="nodep", bufs=3))
    idxp = ctx.enter_context(tc.tile_pool(name="idxp", bufs=2))
    edgep = ctx.enter_context(tc.tile_pool(name="edgep", bufs=3 if V3 else 2))
    smallp = ctx.enter_context(tc.tile_pool(name="smallp", bufs=3))
    ps_o = ctx.enter_context(tc.tile_pool(name="ps_o", bufs=2, space="PSUM"))
    if V2:
        ps_n = ctx.enter_context(
            tc.tile_pool(name="ps_n", bufs=2 if V3 else 3, space="PSUM")
        )
        ps_t = ctx.enter_context(tc.tile_pool(name="ps_t", bufs=1, space="PSUM"))
        ps_me = ctx.enter_context(
            tc.tile_pool(name="ps_me", bufs=3 if V3 else 2, space="PSUM")
        )
    else:
        ps_n = ctx.enter_context(tc.tile_pool(name="ps_n", bufs=2, space="PSUM"))
        ps_t = ctx.enter_context(tc.tile_pool(name="ps_t", bufs=2, space="PSUM"))
    EDT = BF16 if V2 else F32  # edge-phase working dtype
    XDT = BF16 if V2 else F32  # xl table dtype

    # internal DRAM buffers (each tile is its own tensor -> offset 0 for
    # indirect-DMA sources)
    xl_sh = [dram.tile([shard, D], XDT, name=f"xl_sh{l}") for l in range(L)]
    xl_all = [
        dram.tile([cfg.N, D], XDT, name=f"xl_all{l}", addr_space="Shared")
        for l in range(L)
    ]
    xr_dram = [dram.tile([shard, D], F32, name=f"xr_dram{l}") for l in range(L)]
    xst = [dram.tile([shard, D], F32, name=f"xst{l}") for l in range(L - 1)]
    xT = [dram.tile([P, chunks * P], F32, name=f"xT{l}") for l in range(L)]

    # constants resident in SBUF
    iota_sb = consts.tile([P, P], F32, name="iota_sb")
    nc.gpsimd.dma_start(out=iota_sb[:], in_=_row_bcast(io["iota_row"], 0, P, P))
    ident_sb = consts.tile([P, P], F32, name="ident_sb")
    nc.sync.dma_start(out=ident_sb[:], in_=io["ident"][:, :])
    if V2:
        pid_sb = consts.tile([P, 1], BF16, name="pid_sb")
        nc.sync.dma_start(out=pid_sb[:], in_=io["pid_col"][:, :])
        ident16_sb = consts.tile([P, P], BF16, name="ident16_sb")
        nc.sync.dma_start(out=ident16_sb[:], in_=io["ident16"][:, :])
        iota16_sb = consts.tile([P, P], BF16, name="iota16_sb")
        nc.gpsimd.dma_start(out=iota16_sb[:], in_=_row_bcast(io["iota16"], 0, P, P))

    # ------------------------------------------------------------------
    # prologue: build xT[0] = transpose of x_shard
    # ------------------------------------------------------------------
    for t in range(chunks):
        nt = min(P, shard - t * P)
        xq0 = nodep.tile([P, D], F32, name="xq0")
        nc.sync.dma_start(out=xq0[:nt, :], in_=io["x_shard"][t * P : t * P + nt, :])
        psT = ps_t.tile([P, P], F32, name="psT", tag="psT")
        nc.tensor.transpose(
            out=psT[:, :nt], in_=xq0[:nt, :], identity=ident_sb[:nt, :nt]
        )
        sbT = nodep.tile([P, P], F32, name="sbT")
        nc.scalar.activation(out=sbT[:, :nt], in_=psT[:, :nt], func=AF.Copy)
        nc.sync.dma_start(out=xT[0][:, t * P : t * P + nt], in_=sbT[:, :nt])

    L_eff = DBG_LAYERS if DBG_LAYERS else L
    for l in range(L_eff):
        # per-layer constants (broadcast across partitions)
        wl_sb = lconsts.tile([P, D], F32, name="wl_sb")
        nc.sync.dma_start(out=wl_sb[:], in_=io["Wl"][l, :, :])
        wr_sb = lconsts.tile([P, D], F32, name="wr_sb")
        nc.sync.dma_start(out=wr_sb[:], in_=io["Wr"][l, :, :])
        if V2:
            attb_sb = lconsts.tile([P, D], BF16, name="attb_sb")
            nc.gpsimd.dma_start(
                out=attb_sb[:], in_=_row_bcast(io["attb16"], l, P, D)
            )
        else:
            attb_sb = lconsts.tile([P, D], F32, name="attb_sb")
            nc.gpsimd.dma_start(out=attb_sb[:], in_=_row_bcast(io["attb"], l, P, D))
        bc_sb = lconsts.tile([P, D], F32, name="bc_sb")
        nc.gpsimd.dma_start(out=bc_sb[:], in_=_row_bcast(io["bc"], l, P, D))
        cvec_sb = lconsts.tile([P, D], F32, name="cvec_sb")
        nc.gpsimd.dma_start(out=cvec_sb[:], in_=_row_bcast(io["cvec"], l, P, D))
        gamma_sb = lconsts.tile([P, D], F32, name="gamma_sb")
        nc.gpsimd.dma_start(out=gamma_sb[:], in_=_row_bcast(io["gamma"], l, P, D))
        beta_sb = lconsts.tile([P, D], F32, name="beta_sb")
        nc.gpsimd.dma_start(out=beta_sb[:], in_=_row_bcast(io["beta"], l, P, D))

        # --------------------------------------------------------------
        # node phase: xl = x@Wl (shard), xr = x@Wr + (bl+br) (shard)
        # --------------------------------------------------------------
        for t in range(chunks):
            nt = min(P, shard - t * P)
            lhsT = nodep.tile([P, P], F32, name="lhsT")
            nc.sync.dma_start(out=lhsT[:, :nt], in_=xT[l][:, t * P : t * P + nt])
            ps_xl = ps_n.tile([P, D], F32, name="ps_xl", tag="ps_n")
            nc.tensor.matmul(
                out=ps_xl[:nt, :], lhsT=lhsT[:, :nt], rhs=wl_sb[:], start=True, stop=True
            )
            xl_o = nodep.tile([P, D], XDT, name="xl_o")
            nc.scalar.activation(out=xl_o[:nt, :], in_=ps_xl[:nt, :], func=AF.Copy)
            nc.sync.dma_start(out=xl_sh[l][t * P : t * P + nt, :], in_=xl_o[:nt, :])

            ps_xr = ps_n.tile([P, D], F32, name="ps_xr", tag="ps_n")
            nc.tensor.matmul(
                out=ps_xr[:nt, :], lhsT=lhsT[:, :nt], rhs=wr_sb[:], start=True, stop=True
            )
            xr_o = nodep.tile([P, D], F32, name="xr_o")
            nc.vector.tensor_tensor(
                out=xr_o[:nt, :], in0=ps_xr[:nt, :], in1=bc_sb[:nt, :], op=ALU.add
            )
            nc.sync.dma_start(out=xr_dram[l][t * P : t * P + nt, :], in_=xr_o[:nt, :])

        # --------------------------------------------------------------
        # AllGather xl across the 8 cores
        # --------------------------------------------------------------
        nc.gpsimd.collective_compute(
            "AllGather",
            ALU.bypass,
            replica_groups=[list(range(cfg.M))],
            ins=[xl_sh[l][:, :].opt()],
            outs=[xl_all[l][:, :].opt()],
        )

        # --------------------------------------------------------------
        # edge phase, one chunk of 128 destinations at a time
        # --------------------------------------------------------------
        for ch in range(chunks):
            nt = min(P, shard - ch * P)
            rows = slice(ch * P, ch * P + nt)

            if V2:
                dstl_sb = idxp.tile([P, K], BF16, name="dstl_sb")
                nc.sync.dma_start(out=dstl_sb[:], in_=io["dstl16"][ch, :, :])
            else:
                dstl_sb = idxp.tile([P, K], F32, name="dstl_sb")
                nc.sync.dma_start(out=dstl_sb[:], in_=io["dstl"][ch, :, :])

            # m = xr[dst] + xl[src]
            if V2:
                # xl rows gathered (bf16); xr rows via PE segment-broadcast
                g2 = edgep.tile([P, K, D], BF16, name="g2")
                srcg_sb = idxp.tile([P, K], I32, name="srcg_sb")
                nc.sync.dma_start(out=srcg_sb[:], in_=io["src_idx"][ch, :, :])
                _gather(nc, g2, xl_all[l], srcg_sb, K, None)

                xr_ch = smallp.tile([P, D], F32, name="xr_ch")
                nc.sync.dma_start(out=xr_ch[:nt, :], in_=xr_dram[l][rows, :])
                xr16 = smallp.tile([P, D], BF16, name="xr16")
                nc.vector.memset(xr16[:, :], 0.0)
                nc.scalar.activation(
                    out=xr16[:nt, :], in_=xr_ch[:nt, :], func=AF.Copy
                )

                dstlb = edgep.tile([P, K * P], BF16, name="dstlb")
                nc.gpsimd.dma_start(
                    out=dstlb[:], in_=_row_bcast(io["dstl_kt"], ch, P, K * P)
                )
                ST = edgep.tile([P, K * P], BF16, name="ST")
                nc.vector.tensor_tensor(
                    out=ST[:, :],
                    in0=pid_sb[:, :].to_broadcast([P, K * P]),
                    in1=dstlb[:, :],
                    op=ALU.is_equal,
                )

                m_t = edgep.tile([P, K, D], BF16, name="m16")
                for k in range(K):
                    pm = ps_me.tile([P, D], F32, name="pm")
                    if V3:
                        nc.tensor.matmul(
                            out=pm[:, :], lhsT=ST[:, k * P : (k + 1) * P],
                            rhs=xr16[:, :], start=True, stop=True,
                        )
                        nc.vector.tensor_tensor(
                            out=m_t[:, k, :], in0=pm[:, :], in1=g2[:, k, :],
                            op=ALU.add,
                        )
                    else:
                        nc.tensor.matmul(
                            out=pm[:, :], lhsT=ST[:, k * P : (k + 1) * P],
                            rhs=xr16[:, :], start=True, stop=False,
                        )
                        nc.tensor.matmul(
                            out=pm[:, :], lhsT=ident16_sb[:, :], rhs=g2[:, k, :],
                            start=False, stop=True,
                        )
                        nc.scalar.activation(
                            out=m_t[:, k, :], in_=pm[:, :], func=AF.Copy
                        )
            elif SPLIT_GATHER:
                m_t = edgep.tile([P, K, D], F32, name="m_t")
                g2 = edgep.tile([P, K, D], F32, name="g2")
                dstg_sb = idxp.tile([P, K], I32, name="dstg_sb")
                nc.sync.dma_start(out=dstg_sb[:], in_=io["dstg"][ch, :, :])
                srcg_sb = idxp.tile([P, K], I32, name="srcg_sb")
                nc.sync.dma_start(out=srcg_sb[:], in_=io["src_idx"][ch, :, :])
                _gather(nc, m_t, xr_dram[l], dstg_sb, K, None)
                _gather(nc, g2, xl_all[l], srcg_sb, K, None)
            else:
                m_t = edgep.tile([P, K, D], F32, name="m_t")
                g2 = edgep.tile([P, K, D], F32, name="g2")
                idxd_sb = idxp.tile([P, K * 8], I16, name="idxd_sb")
                nc.sync.dma_start(out=idxd_sb[:], in_=io["idx_dst"][ch, :, :])
                idxl_sb = idxp.tile([P, KLO * 8], I16, name="idxl_sb")
                nc.sync.dma_start(out=idxl_sb[:], in_=io["idx_lo"][ch, :, :])
                idxh_sb = idxp.tile([P, KHI * 8], I16, name="idxh_sb")
                nc.sync.dma_start(out=idxh_sb[:], in_=io["idx_hi"][ch, :, :])
                nc.gpsimd.dma_gather(
                    out_ap=m_t[:, :, :],
                    in_ap=xr_dram[l][:, :],
                    idxs_ap=idxd_sb[:, :],
                    num_idxs=K * P,
                    num_idxs_reg=K * P,
                    elem_size=D,
                )
                nc.gpsimd.dma_gather(
                    out_ap=g2[:, 0:KLO, :],
                    in_ap=xl_all[l][0:HALF, :],
                    idxs_ap=idxl_sb[:, :],
                    num_idxs=KLO * P,
                    num_idxs_reg=KLO * P,
                    elem_size=D,
                )
                nc.gpsimd.dma_gather(
                    out_ap=g2[:, KLO:K, :],
                    in_ap=xl_all[l][HALF : cfg.N, :],
                    idxs_ap=idxh_sb[:, :],
                    num_idxs=KHI * P,
                    num_idxs_reg=KHI * P,
                    elem_size=D,
                )
            if not V2:
                nc.vector.tensor_tensor(
                    out=m_t[:, :, :], in0=m_t[:, :, :], in1=g2[:, :, :],
                    op=ALU.add,
                )

            # selection matrix S[e, dst_local]
            S = edgep.tile([P, K, P], EDT, name="S")
            nc.vector.tensor_tensor(
                out=S[:, :, :],
                in0=dstl_sb[:, :].unsqueeze(2).to_broadcast([P, K, P]),
                in1=(iota16_sb if V2 else iota_sb)[:, :]
                .unsqueeze(1)
                .to_broadcast([P, K, P]),
                op=ALU.is_equal,
            )

            # leaky relu, attention logits, exp
            lk = edgep.tile([P, K, D], EDT, name="lk")
            if USE_ACT_LRELU:
                nc.scalar.activation(
                    out=lk[:, :, :], in_=m_t[:, :, :], func=AF.Lrelu, alpha=NEG_SLOPE
                )
            else:
                # leaky(x) = max(x, 0.2*x)
                nc.vector.tensor_scalar(
                    out=lk[:, :, :], in0=m_t[:, :, :], scalar1=NEG_SLOPE,
                    scalar2=None, op0=ALU.mult,
                )
                nc.vector.tensor_tensor(
                    out=lk[:, :, :], in0=lk[:, :, :], in1=m_t[:, :, :], op=ALU.max
                )
            tt = edgep.tile([P, K, D], EDT, name="tt")
            nc.vector.tensor_tensor(
                out=tt[:, :, :],
                in0=lk[:, :, :],
                in1=attb_sb[:, :].unsqueeze(1).to_broadcast([P, K, D]),
                op=ALU.mult,
            )
            lg = smallp.tile([P, K, H], F32, name="lg")
            nc.vector.reduce_sum(
                out=lg[:, :, :],
                in_=tt[:, :, :].rearrange("p k (h c) -> p k h c", h=H),
                axis=AX.X,
            )
            zee = edgep.tile([P, K, D + H], EDT, name="zee")
            nc.scalar.activation(out=zee[:, :, D : D + H], in_=lg[:, :, :], func=AF.Exp)
            nc.vector.tensor_tensor(
                out=zee[:, :, 0:D].rearrange("p k (h c) -> p k h c", h=H),
                in0=m_t[:, :, :].rearrange("p k (h c) -> p k h c", h=H),
                in1=zee[:, :, D : D + H].unsqueeze(3).to_broadcast([P, K, H, C]),
                op=ALU.mult,
            )

            # segment sums on PE: psum[dst, 0:D] = sum ee*m ; psum[dst, D:D+H] = denom
            po = ps_o.tile([P, D + H], F32, name="po")
            for k in range(K):
                nc.tensor.matmul(
                    out=po[:, :],
                    lhsT=S[:, k, :],
                    rhs=zee[:, k, :],
                    start=(k == 0),
                    stop=(k == K - 1),
                )

            dn = smallp.tile([P, H], F32, name="dn")
            nc.vector.tensor_scalar(
                out=dn[:, :], in0=po[:, D : D + H], scalar1=DENOM_EPS, scalar2=None,
                op0=ALU.add,
            )
            rd = smallp.tile([P, H], F32, name="rd")
            nc.vector.reciprocal(out=rd[:, :], in_=dn[:, :])

            onrm = smallp.tile([P, D], F32, name="onrm")
            nc.vector.tensor_tensor(
                out=onrm[:, :].rearrange("p (h c) -> p h c", h=H),
                in0=po[:, 0:D].rearrange("p (h c) -> p h c", h=H),
                in1=rd[:, :].unsqueeze(2).to_broadcast([P, H, C]),
                op=ALU.mult,
            )

            # h = onrm - xr[dst] + (bl + gat_bias); then residual + LN
            if not V2:
                xr_ch = smallp.tile([P, D], F32, name="xr_ch")
                nc.sync.dma_start(out=xr_ch[:nt, :], in_=xr_dram[l][rows, :])
            xq = smallp.tile([P, D], F32, name="xq")
            if l == 0:
                nc.sync.dma_start(out=xq[:nt, :], in_=io["x_shard"][rows, :])
            else:
                nc.sync.dma_start(out=xq[:nt, :], in_=xst[l - 1][rows, :])

            t1 = smallp.tile([P, D], F32, name="t1")
            nc.vector.tensor_tensor(
                out=t1[:nt, :], in0=onrm[:nt, :], in1=xr_ch[:nt, :], op=ALU.subtract
            )
            t2 = smallp.tile([P, D], F32, name="t2")
            nc.vector.tensor_tensor(
                out=t2[:nt, :], in0=t1[:nt, :], in1=cvec_sb[:nt, :], op=ALU.add
            )
            t3 = smallp.tile([P, D], F32, name="t3")
            nc.vector.tensor_tensor(
                out=t3[:nt, :], in0=t2[:nt, :], in1=xq[:nt, :], op=ALU.add
            )

            st6 = smallp.tile([P, 6], F32, name="st6")
            nc.vector.bn_stats(out=st6[:nt, :], in_=t3[:nt, :])
            mv = smallp.tile([P, 2], F32, name="mv")
            nc.vector.bn_aggr(out=mv[:nt, :], in_=st6[:nt, :])
            veps = smallp.tile([P, 1], F32, name="veps")
            nc.vector.tensor_scalar(
                out=veps[:nt, :], in0=mv[:nt, 1:2], scalar1=LN_EPS, scalar2=None,
                op0=ALU.add,
            )
            sd = smallp.tile([P, 1], F32, name="sd")
            nc.scalar.activation(out=sd[:nt, :], in_=veps[:nt, :], func=AF.Sqrt)
            rstd = smallp.tile([P, 1], F32, name="rstd")
            nc.vector.reciprocal(out=rstd[:nt, :], in_=sd[:nt, :])

            y1 = smallp.tile([P, D], F32, name="y1")
            nc.vector.tensor_scalar(
                out=y1[:nt, :], in0=t3[:nt, :], scalar1=mv[:nt, 0:1],
                scalar2=rstd[:nt, :], op0=ALU.subtract, op1=ALU.mult,
            )
            y2 = smallp.tile([P, D], F32, name="y2")
            nc.vector.tensor_tensor(
                out=y2[:nt, :], in0=y1[:nt, :], in1=gamma_sb[:nt, :], op=ALU.mult
            )
            y3 = smallp.tile([P, D], F32, name="y3")
            nc.vector.tensor_tensor(
                out=y3[:nt, :], in0=y2[:nt, :], in1=beta_sb[:nt, :], op=ALU.add
            )

            if l < L_eff - 1:
                xo = smallp.tile([P, D], F32, name="xo")
                nc.scalar.activation(out=xo[:nt, :], in_=y3[:nt, :], func=AF.Relu)
                nc.sync.dma_start(out=xst[l][rows, :], in_=xo[:nt, :])
                psT2 = ps_t.tile([P, P], F32, name="psT2", tag="psT")
                nc.tensor.transpose(
                    out=psT2[:, :nt], in_=xo[:nt, :], identity=ident_sb[:nt, :nt]
                )
                sbT2 = smallp.tile([P, P], F32, name="sbT2")
                nc.scalar.activation(out=sbT2[:, :nt], in_=psT2[:, :nt], func=AF.Copy)
                nc.sync.dma_start(
                    out=xT[l + 1][:, ch * P : ch * P + nt], in_=sbT2[:, :nt]
                )
            else:
                nc.sync.dma_start(out=io["y"][rows, :], in_=y3[:nt, :])

    ctx.close()


def _gather(nc, out_tile, src_dram, idx_sb, K, op):
    """Indirect row gather DRAM->SBUF. out_tile [P, K, D]; idx [P, K]."""
    kw = {} if op is None else {"compute_op": op}
    if SPLIT_GATHER:
        for k in range(K):
            nc.gpsimd.indirect_dma_start(
                out=out_tile[:, k, :],
                out_offset=None,
                in_=src_dram[:, :],
                in_offset=IndirectOffsetOnAxis(ap=idx_sb[:, k : k + 1], axis=0),
                **kw,
            )
    else:
        nc.gpsimd.indirect_dma_start(
            out=out_tile[:, :, :],
            out_offset=None,
            in_=src_dram[:, :],
            in_offset=IndirectOffsetOnAxis(ap=idx_sb[:, :], axis=0),
            **kw,
        )


def _row_bcast(ap, row, parts, d):
    """AP reading row `row` of a [R, 1, D] or [R, D] DRAM tensor, replicated
    across `parts` partitions (partition step 0)."""
    flat = ap[row] if ap.ndim == 3 else ap[row : row + 1]
    base = flat.opt()
    return bass.AP(tensor=base.tensor, offset=row * d, ap=[[0, parts], [1, d]])


# ----------------------------------------------------------------------------
# host-side inputs
# ----------------------------------------------------------------------------

def make_host_inputs(inputs, cfg):
    L, D, H, C = cfg.L, cfg.D, cfg.H, cfg.C
    Wl = np.asarray(inputs["Wl"], np.float32)
    Wr = np.asarray(inputs["Wr"], np.float32)
    bl = np.asarray(inputs["bl"], np.float32)
    br = np.asarray(inputs["br"], np.float32)
    att = np.asarray(inputs["att"], np.float32)
    gat_bias = np.asarray(inputs["bias"], np.float32)
    gamma = np.asarray(inputs["gamma"], np.float32)
    beta = np.asarray(inputs["beta"], np.float32)
    return {
        "Wl": Wl,
        "Wr": Wr,
        "attb": att.reshape(L, 1, H * C),
        "bc": (bl + br).reshape(L, 1, D),
        "cvec": (bl + gat_bias).reshape(L, 1, D),
        "gamma": gamma.reshape(L, 1, D),
        "beta": beta.reshape(L, 1, D),
        "iota_row": np.arange(P, dtype=np.float32).reshape(1, P),
        "ident": np.eye(P, dtype=np.float32),
        "attb16": att.reshape(L, 1, H * C).astype(ml_dtypes.bfloat16),
        "iota16": np.arange(P, dtype=np.float32)
        .reshape(1, P)
        .astype(ml_dtypes.bfloat16),
        "pid_col": np.arange(P, dtype=np.float32)
        .reshape(P, 1)
        .astype(ml_dtypes.bfloat16),
        "ident16": np.eye(P, dtype=np.float32).astype(ml_dtypes.bfloat16),
    }


def make_in_maps(inputs, pre, cfg):
    x = np.asarray(inputs["fine_poi_x"], np.float32)
    shared = make_host_inputs(inputs, cfg)
    in_maps = []
    for c in range(cfg.M):
        m = dict(shared)
        m["x_shard"] = np.ascontiguousarray(
            x[c * cfg.shard : (c + 1) * cfg.shard]
        )
        for k in (
            "src_idx", "dstg", "dstl", "idx_lo", "idx_hi", "idx_dst",
            "dstl16", "dstl_kt",
        ):
            m[k] = pre[c][k]
        in_maps.append(m)
    return in_maps


# ----------------------------------------------------------------------------
# program assembly + execution
# ----------------------------------------------------------------------------

_CACHE = {}


def _build_program(cfg, meta):
    K = meta["K"]
    key = (cfg.N, cfg.D, cfg.H, cfg.L, cfg.M, K, meta["KLO"])
    if key in _CACHE:
        return _CACHE[key]
    nc = bacc.Bacc(
        "TRN2", target_bir_lowering=False, debug=False, num_devices=cfg.M
    )
    io = {}
    io["x_shard"] = nc.dram_tensor(
        "x_shard", [cfg.shard, cfg.D], F32, kind="ExternalInput"
    ).ap()
    if SPLIT_GATHER:
        io["src_idx"] = nc.dram_tensor(
            "src_idx", [cfg.chunks, P, K], I32, kind="ExternalInput"
        ).ap()
        io["dstg"] = nc.dram_tensor(
            "dstg", [cfg.chunks, P, K], I32, kind="ExternalInput"
        ).ap()
    else:
        io["idx_lo"] = nc.dram_tensor(
            "idx_lo", [cfg.chunks, P, meta["KLO"] * 8], I16, kind="ExternalInput"
        ).ap()
        io["idx_hi"] = nc.dram_tensor(
            "idx_hi", [cfg.chunks, P, meta["KHI"] * 8], I16, kind="ExternalInput"
        ).ap()
        io["idx_dst"] = nc.dram_tensor(
            "idx_dst", [cfg.chunks, P, K * 8], I16, kind="ExternalInput"
        ).ap()
    io["dstl"] = nc.dram_tensor(
        "dstl", [cfg.chunks, P, K], F32, kind="ExternalInput"
    ).ap()
    if V2:
        io["dstl16"] = nc.dram_tensor(
            "dstl16", [cfg.chunks, P, K], BF16, kind="ExternalInput"
        ).ap()
        io["dstl_kt"] = nc.dram_tensor(
            "dstl_kt", [cfg.chunks, 1, K * P], BF16, kind="ExternalInput"
        ).ap()
        io["attb16"] = nc.dram_tensor(
            "attb16", [cfg.L, 1, cfg.D], BF16, kind="ExternalInput"
        ).ap()
        io["iota16"] = nc.dram_tensor(
            "iota16", [1, P], BF16, kind="ExternalInput"
        ).ap()
        io["pid_col"] = nc.dram_tensor(
            "pid_col", [P, 1], BF16, kind="ExternalInput"
        ).ap()
        io["ident16"] = nc.dram_tensor(
            "ident16", [P, P], BF16, kind="ExternalInput"
        ).ap()
    io["Wl"] = nc.dram_tensor(
        "Wl", [cfg.L, cfg.D, cfg.D], F32, kind="ExternalInput"
    ).ap()
    io["Wr"] = nc.dram_tensor(
        "Wr", [cfg.L, cfg.D, cfg.D], F32, kind="ExternalInput"
    ).ap()
    for nm in ["attb", "bc", "cvec", "gamma", "beta"]:
        io[nm] = nc.dram_tensor(
            nm, [cfg.L, 1, cfg.D], F32, kind="ExternalInput"
        ).ap()
    io["iota_row"] = nc.dram_tensor(
        "iota_row", [1, P], F32, kind="ExternalInput"
    ).ap()
    io["ident"] = nc.dram_tensor("ident", [P, P], F32, kind="ExternalInput").ap()
    io["y"] = nc.dram_tensor(
        "y", [cfg.shard, cfg.D], F32, kind="ExternalOutput"
    ).ap()

    with tile.TileContext(nc) as tc:
        build(tc, io, cfg, meta)
    nc.compile()
    _CACHE[key] = nc
    return nc


def kernel(**inputs):
    from concourse import bass_utils

    cfg = Cfg()
    pre, meta = preprocess(inputs["edge_index"], cfg)
    nc = _build_program(cfg, meta)
    in_maps = make_in_maps(inputs, pre, cfg)
    res = bass_utils.run_bass_kernel_spmd(
        nc, in_maps, core_ids=list(range(cfg.M))
    )
    out = np.concatenate([res.results[c]["y"] for c in range(cfg.M)], axis=0)
    return out.astype(np.float32)



# revision 4
# speedup vs baseline: 1.2554x; 1.2554x over previous
"""GATv2 (3-layer, 4-head) message-passing kernel for Trainium2, 8-core SPMD.

Strategy (per sharding hint): nodes sharded contiguously across 8 cores;
edges partitioned by destination; per-layer AllGather of the source-side
transform xl = x @ Wl (bf16) so each core can gather arbitrary source rows;
segment softmax / scatter-add stay local per destination shard.

Algebra: with m_e = xl[src_e] + xr[dst_e] (xr = x@Wr + (bl+br)) and
ee = exp(att . leaky(m)), the attention output per destination is
    sum_e ee_e * m_e / denom - xr[dst] + (bl + gat_bias)
so only one row-gather per edge (xl) is needed; the xr side is a
PE segment-broadcast (ST_k^T @ xr_chunk).

v2 changes vs the 5.6ms baseline (HW-measured):
- selection matrices S (scatter) and ST (broadcast) are static per edge
  structure -> precomputed on host, streamed from DRAM on the scalar/sync
  hwdge queues (no on-chip is_equal builds, no gpsimd broadcast DMAs).
- per-chunk K is variable (max over cores) -> ~6% fewer indirect gathers;
  the indirect gather is the hard bottleneck at ~1.05us/instruction fixed.
- all K ST-matmuls accumulate into one 5-bank PSUM region; a single DVE
  add (+ gathered xl) builds m (replaces 19 tiny adds).
- leaky relu is one scalar-engine Prelu (AP alpha); rstd = exp(-0.5*ln(v))
  keeps every scalar activation in one table set (no ACT_TABLE thrash).
- x (residual, f32), xT (bf16 lhsT), xr (bf16) are SBUF-resident.
"""

import os
import sys

sys.path.insert(0, "/opt/trn_rl_repo")

import ml_dtypes
import numpy as np

import concourse.bass as bass
import concourse.bacc as bacc
import concourse.tile as tile
from concourse import mybir
from concourse.bass import IndirectOffsetOnAxis

F32 = mybir.dt.float32
BF16 = mybir.dt.bfloat16
I32 = mybir.dt.int32
AF = mybir.ActivationFunctionType
ALU = mybir.AluOpType
AX = mybir.AxisListType

P = 128
NEG_SLOPE = 0.2
LN_EPS = 1e-5
DENOM_EPS = 1e-30
DBG_LAYERS = int(os.environ.get("GAT_LAYERS", "0"))  # 0 = all


class Cfg:
    def __init__(self, N=50000, D=128, H=4, L=3, n_cores=8):
        self.N, self.D, self.H, self.L, self.M = N, D, H, L, n_cores
        self.C = D // H
        assert N % n_cores == 0
        self.shard = N // n_cores
        self.chunks = (self.shard + P - 1) // P


# ----------------------------------------------------------------------------
# Host preprocessing: append self loops, sort by dst, pack per-core chunk
# edge lists, build S / ST selection matrices.
# ----------------------------------------------------------------------------

def preprocess(edge_index, cfg):
    N, M, shard, chunks = cfg.N, cfg.M, cfg.shard, cfg.chunks
    ei = np.asarray(edge_index)
    loops = np.arange(N, dtype=np.int64)
    src = np.concatenate([ei[0].astype(np.int64), loops])
    dst = np.concatenate([ei[1].astype(np.int64), loops])
    order = np.argsort(dst, kind="stable")
    src_s, dst_s = src[order], dst[order]

    # per-(core, chunk) edge lists
    per_core = []
    cnts = np.zeros((M, chunks), dtype=np.int64)
    for c in range(M):
        lo, hi = np.searchsorted(dst_s, [c * shard, (c + 1) * shard])
        d_loc = dst_s[lo:hi] - c * shard
        s_loc = src_s[lo:hi]
        ch = d_loc // P
        chunk_edges = []
        for t in range(chunks):
            m = ch == t
            chunk_edges.append((s_loc[m], (d_loc[m] - t * P)))
            cnts[c, t] = int(m.sum())
        per_core.append(chunk_edges)

    K_list = [int(-(-cnts[:, t].max() // P)) for t in range(chunks)]
    offs = np.concatenate([[0], np.cumsum(K_list)]).astype(np.int64)
    TOTK = int(offs[-1])

    pre = []
    for c in range(M):
        src_idx = np.zeros((P, TOTK), dtype=np.int32)
        S = np.zeros((P, TOTK * P), dtype=ml_dtypes.bfloat16)
        ST = np.zeros((P, TOTK * P), dtype=ml_dtypes.bfloat16)
        for t in range(chunks):
            sl, dl = per_core[c][t]
            j = np.arange(len(sl))
            p, k = j % P, j // P
            src_idx[p, offs[t] + k] = sl
            # S[p_slot, (off+k)*P + d] = 1 ; ST[d, (off+k)*P + p_slot] = 1
            S[p, (offs[t] + k) * P + dl] = 1
            ST[dl, (offs[t] + k) * P + p] = 1
        pre.append({"src_idx": src_idx, "S": S, "ST": ST})
    meta = {"K_list": K_list, "offs": offs, "TOTK": TOTK}
    return pre, meta


# ----------------------------------------------------------------------------
# Kernel builder
# ----------------------------------------------------------------------------

def build(tc, io, cfg, meta):
    from contextlib import ExitStack

    nc = tc.nc
    D, H, L, C = cfg.D, cfg.H, cfg.L, cfg.C
    shard, chunks = cfg.shard, cfg.chunks
    K_list, offs = meta["K_list"], meta["offs"]

    ctx = ExitStack()
    dram = ctx.enter_context(tc.tile_pool(name="drampool", bufs=1, space="DRAM"))
    consts = ctx.enter_context(tc.tile_pool(name="consts", bufs=1))
    state = ctx.enter_context(tc.tile_pool(name="state", bufs=1))
    lconsts = ctx.enter_context(tc.tile_pool(name="lconsts", bufs=2))
    nodep = ctx.enter_context(tc.tile_pool(name="nodep", bufs=3))
    idxp = ctx.enter_context(tc.tile_pool(name="idxp", bufs=3))
    edgep = ctx.enter_context(tc.tile_pool(name="edgep", bufs=2))
    smallp = ctx.enter_context(tc.tile_pool(name="smallp", bufs=3))
    ps_big = ctx.enter_context(tc.tile_pool(name="ps_big", bufs=1, space="PSUM"))
    ps_o = ctx.enter_context(tc.tile_pool(name="ps_o", bufs=1, space="PSUM"))
    ps_n = ctx.enter_context(tc.tile_pool(name="ps_n", bufs=1, space="PSUM"))
    ps_t = ctx.enter_context(tc.tile_pool(name="ps_t", bufs=1, space="PSUM"))

    KMAX = max(K_list)

    # internal DRAM
    xl_sh = [dram.tile([shard, D], BF16, name=f"xl_sh{l}") for l in range(L)]
    xl_all = [
        dram.tile([cfg.N, D], BF16, name=f"xl_all{l}", addr_space="Shared")
        for l in range(L)
    ]

    # SBUF-resident constants / state
    ident_sb = consts.tile([P, P], F32, name="ident_sb")
    nc.sync.dma_start(out=ident_sb[:], in_=io["ident"][:, :])
    alpha_sb = consts.tile([P, 1], F32, name="alpha_sb")
    nc.vector.memset(alpha_sb[:, :], NEG_SLOPE)
    x_sb = state.tile([P, chunks, D], F32, name="x_sb")
    xT_sb = state.tile([P, chunks * P], BF16, name="xT_sb")
    xr_sb = state.tile([P, chunks, D], BF16, name="xr_sb")
    nc.vector.memset(xr_sb[:, :, :], 0.0)

    # ------------------------------------------------------------------
    # prologue: x -> x_sb; xT_sb = transpose(x) (bf16)
    # ------------------------------------------------------------------
    for t in range(chunks):
        nt = min(P, shard - t * P)
        nc.sync.dma_start(
            out=x_sb[:nt, t, :], in_=io["x_shard"][t * P : t * P + nt, :]
        )
    for t in range(chunks):
        nt = min(P, shard - t * P)
        psT = ps_t.tile([P, P], F32, name="psT", tag="psT")
        nc.tensor.transpose(
            out=psT[:, :nt], in_=x_sb[:nt, t, :], identity=ident_sb[:nt, :nt]
        )
        nc.scalar.activation(
            out=xT_sb[:, t * P : t * P + nt], in_=psT[:, :nt], func=AF.Copy
        )

    L_eff = DBG_LAYERS if DBG_LAYERS else L
    for l in range(L_eff):
        # per-layer constants
        wl_sb = lconsts.tile([P, D], BF16, name="wl_sb")
        nc.sync.dma_start(out=wl_sb[:], in_=io["Wl16"][l, :, :])
        wr_sb = lconsts.tile([P, D], BF16, name="wr_sb")
        nc.sync.dma_start(out=wr_sb[:], in_=io["Wr16"][l, :, :])
        attb_sb = lconsts.tile([P, D], BF16, name="attb_sb")
        nc.gpsimd.dma_start(out=attb_sb[:], in_=_row_bcast(io["attb16"], l, P, D))
        bc_sb = lconsts.tile([P, D], F32, name="bc_sb")
        nc.gpsimd.dma_start(out=bc_sb[:], in_=_row_bcast(io["bc"], l, P, D))
        cvec_sb = lconsts.tile([P, D], F32, name="cvec_sb")
        nc.gpsimd.dma_start(out=cvec_sb[:], in_=_row_bcast(io["cvec"], l, P, D))
        gamma_sb = lconsts.tile([P, D], F32, name="gamma_sb")
        nc.gpsimd.dma_start(out=gamma_sb[:], in_=_row_bcast(io["gamma"], l, P, D))
        beta_sb = lconsts.tile([P, D], F32, name="beta_sb")
        nc.gpsimd.dma_start(out=beta_sb[:], in_=_row_bcast(io["beta"], l, P, D))

        # --------------------------------------------------------------
        # node phase: xl = x@Wl (bf16 -> DRAM), xr = x@Wr + bc (bf16, SBUF)
        # --------------------------------------------------------------
        for t in range(chunks):
            nt = min(P, shard - t * P)
            lhsT = xT_sb[:, t * P : t * P + nt]
            ps_xl = ps_n.tile([P, D], F32, name="ps_xl", tag="ps_n")
            nc.tensor.matmul(
                out=ps_xl[:nt, :], lhsT=lhsT, rhs=wl_sb[:], start=True, stop=True
            )
            xl_o = nodep.tile([P, D], BF16, name="xl_o")
            nc.scalar.activation(out=xl_o[:nt, :], in_=ps_xl[:nt, :], func=AF.Copy)
            nc.sync.dma_start(out=xl_sh[l][t * P : t * P + nt, :], in_=xl_o[:nt, :])

            ps_xr = ps_n.tile([P, D], F32, name="ps_xr", tag="ps_n")
            nc.tensor.matmul(
                out=ps_xr[:nt, :], lhsT=lhsT, rhs=wr_sb[:], start=True, stop=True
            )
            nc.vector.tensor_tensor(
                out=xr_sb[:nt, t, :], in0=ps_xr[:nt, :], in1=bc_sb[:nt, :],
                op=ALU.add,
            )

        # --------------------------------------------------------------
        # AllGather xl across the 8 cores
        # --------------------------------------------------------------
        nc.gpsimd.collective_compute(
            "AllGather",
            ALU.bypass,
            replica_groups=[list(range(cfg.M))],
            ins=[xl_sh[l][:, :].opt()],
            outs=[xl_all[l][:, :].opt()],
        )

        # --------------------------------------------------------------
        # edge phase
        # --------------------------------------------------------------
        for ch in range(chunks):
            nt = min(P, shard - ch * P)
            K = K_list[ch]
            off = int(offs[ch])

            idx_sb = idxp.tile([P, KMAX], I32, name="idx_sb")
            nc.sync.dma_start(out=idx_sb[:, :K], in_=io["src_idx"][:, off : off + K])
            S_sb = edgep.tile([P, KMAX * P], BF16, name="S_sb")
            nc.scalar.dma_start(
                out=S_sb[:, : K * P], in_=io["S"][:, off * P : (off + K) * P]
            )
            ST_sb = edgep.tile([P, KMAX * P], BF16, name="ST_sb")
            nc.sync.dma_start(
                out=ST_sb[:, : K * P], in_=io["ST"][:, off * P : (off + K) * P]
            )

            # gather xl rows (the bottleneck: one indirect DMA per k-tile)
            g2 = edgep.tile([P, KMAX, D], BF16, name="g2")
            for k in range(K):
                nc.gpsimd.indirect_dma_start(
                    out=g2[:, k, :],
                    out_offset=None,
                    in_=xl_all[l][:, :],
                    in_offset=IndirectOffsetOnAxis(ap=idx_sb[:, k : k + 1], axis=0),
                )

            # m = (ST_k^T @ xr_chunk) + g2 : K matmuls into one PSUM region,
            # then a single DVE add
            pm = ps_big.tile([P, KMAX * D], F32, name="pm", tag="pm")
            for k in range(K):
                nc.tensor.matmul(
                    out=pm[:, k * D : (k + 1) * D],
                    lhsT=ST_sb[:, k * P : (k + 1) * P],
                    rhs=xr_sb[:, ch, :],
                    start=True,
                    stop=True,
                )
            m_t = edgep.tile([P, KMAX, D], BF16, name="m_t")
            nc.vector.tensor_tensor(
                out=m_t[:, :K, :].rearrange("p k d -> p (k d)"),
                in0=pm[:, : K * D],
                in1=g2[:, :K, :].rearrange("p k d -> p (k d)"),
                op=ALU.add,
            )

            # leaky relu on the scalar engine (Prelu with AP alpha)
            lk = edgep.tile([P, KMAX, D], BF16, name="lk")
            nc.scalar.activation(
                out=lk[:, :K, :].rearrange("p k d -> p (k d)"),
                in_=m_t[:, :K, :].rearrange("p k d -> p (k d)"),
                func=AF.Prelu,
                alpha=alpha_sb[:, 0:1],
            )

            # attention logits and exp
            tt = edgep.tile([P, KMAX, D], BF16, name="tt")
            nc.vector.tensor_tensor(
                out=tt[:, :K, :],
                in0=lk[:, :K, :],
                in1=attb_sb[:, :].unsqueeze(1).to_broadcast([P, K, D]),
                op=ALU.mult,
            )
            lg = smallp.tile([P, KMAX, H], F32, name="lg")
            nc.vector.reduce_sum(
                out=lg[:, :K, :],
                in_=tt[:, :K, :].rearrange("p k (h c) -> p k h c", h=H),
                axis=AX.X,
            )
            zee = edgep.tile([P, KMAX, D + H], BF16, name="zee")
            nc.scalar.activation(
                out=zee[:, :K, D : D + H], in_=lg[:, :K, :], func=AF.Exp
            )
            nc.vector.tensor_tensor(
                out=zee[:, :K, 0:D].rearrange("p k (h c) -> p k h c", h=H),
                in0=m_t[:, :K, :].rearrange("p k (h c) -> p k h c", h=H),
                in1=zee[:, :K, D : D + H].unsqueeze(3).to_broadcast([P, K, H, C]),
                op=ALU.mult,
            )

            # segment sums on PE: po[d, 0:D] = sum ee*m ; po[d, D:D+H] = denom
            po = ps_o.tile([P, D + H], F32, name="po", tag="po")
            for k in range(K):
                nc.tensor.matmul(
                    out=po[:, :],
                    lhsT=S_sb[:, k * P : (k + 1) * P],
                    rhs=zee[:, k, :],
                    start=(k == 0),
                    stop=(k == K - 1),
                )

            # normalize, subtract xr, add cvec, residual, LN
            dn = smallp.tile([P, H], F32, name="dn")
            nc.vector.tensor_scalar(
                out=dn[:, :], in0=po[:, D : D + H], scalar1=DENOM_EPS,
                scalar2=None, op0=ALU.add,
            )
            rd = smallp.tile([P, H], F32, name="rd")
            nc.vector.reciprocal(out=rd[:, :], in_=dn[:, :])
            onrm = smallp.tile([P, D], F32, name="onrm")
            nc.vector.tensor_tensor(
                out=onrm[:, :].rearrange("p (h c) -> p h c", h=H),
                in0=po[:, 0:D].rearrange("p (h c) -> p h c", h=H),
                in1=rd[:, :].unsqueeze(2).to_broadcast([P, H, C]),
                op=ALU.mult,
            )
            t1 = smallp.tile([P, D], F32, name="t1")
            nc.vector.tensor_tensor(
                out=t1[:nt, :], in0=onrm[:nt, :], in1=xr_sb[:nt, ch, :],
                op=ALU.subtract,
            )
            t2 = smallp.tile([P, D], F32, name="t2")
            nc.vector.tensor_tensor(
                out=t2[:nt, :], in0=t1[:nt, :], in1=cvec_sb[:nt, :], op=ALU.add
            )
            t3 = smallp.tile([P, D], F32, name="t3")
            nc.vector.tensor_tensor(
                out=t3[:nt, :], in0=t2[:nt, :], in1=x_sb[:nt, ch, :], op=ALU.add
            )

            st6 = smallp.tile([P, 6], F32, name="st6")
            nc.vector.bn_stats(out=st6[:nt, :], in_=t3[:nt, :])
            mv = smallp.tile([P, 2], F32, name="mv")
            nc.vector.bn_aggr(out=mv[:nt, :], in_=st6[:nt, :])
            veps = smallp.tile([P, 1], F32, name="veps")
            nc.vector.tensor_scalar(
                out=veps[:nt, :], in0=mv[:nt, 1:2], scalar1=LN_EPS, scalar2=None,
                op0=ALU.add,
            )
            lnv = smallp.tile([P, 1], F32, name="lnv")
            nc.scalar.activation(out=lnv[:nt, :], in_=veps[:nt, :], func=AF.Ln)
            rstd = smallp.tile([P, 1], F32, name="rstd")
            nc.scalar.activation(
                out=rstd[:nt, :], in_=lnv[:nt, :], func=AF.Exp, scale=-0.5
            )

            y1 = smallp.tile([P, D], F32, name="y1")
            nc.vector.tensor_scalar(
                out=y1[:nt, :], in0=t3[:nt, :], scalar1=mv[:nt, 0:1],
                scalar2=rstd[:nt, :], op0=ALU.subtract, op1=ALU.mult,
            )
            y2 = smallp.tile([P, D], F32, name="y2")
            nc.vector.tensor_tensor(
                out=y2[:nt, :], in0=y1[:nt, :], in1=gamma_sb[:nt, :], op=ALU.mult
            )

            if l < L_eff - 1:
                y3 = smallp.tile([P, D], F32, name="y3")
                nc.vector.tensor_tensor(
                    out=y3[:nt, :], in0=y2[:nt, :], in1=beta_sb[:nt, :], op=ALU.add
                )
                nc.scalar.activation(
                    out=x_sb[:nt, ch, :], in_=y3[:nt, :], func=AF.Relu
                )
                psT2 = ps_t.tile([P, P], F32, name="psT2", tag="psT")
                nc.tensor.transpose(
                    out=psT2[:, :nt], in_=x_sb[:nt, ch, :],
                    identity=ident_sb[:nt, :nt],
                )
                nc.scalar.activation(
                    out=xT_sb[:, ch * P : ch * P + nt], in_=psT2[:, :nt],
                    func=AF.Copy,
                )
            else:
                y3 = smallp.tile([P, D], F32, name="y3")
                nc.vector.tensor_tensor(
                    out=y3[:nt, :], in0=y2[:nt, :], in1=beta_sb[:nt, :], op=ALU.add
                )
                nc.sync.dma_start(
                    out=io["y"][ch * P : ch * P + nt, :], in_=y3[:nt, :]
                )

    ctx.close()


def _row_bcast(ap, row, parts, d):
    """AP reading row `row` of a [R, 1, D] DRAM tensor, replicated across
    `parts` partitions (partition step 0)."""
    flat = ap[row] if ap.ndim == 3 else ap[row : row + 1]
    base = flat.opt()
    return bass.AP(tensor=base.tensor, offset=row * d, ap=[[0, parts], [1, d]])


# ----------------------------------------------------------------------------
# host-side inputs
# ----------------------------------------------------------------------------

def make_host_inputs(inputs, cfg):
    L, D, H, C = cfg.L, cfg.D, cfg.H, cfg.C
    Wl = np.asarray(inputs["Wl"], np.float32)
    Wr = np.asarray(inputs["Wr"], np.float32)
    bl = np.asarray(inputs["bl"], np.float32)
    br = np.asarray(inputs["br"], np.float32)
    att = np.asarray(inputs["att"], np.float32)
    gat_bias = np.asarray(inputs["bias"], np.float32)
    gamma = np.asarray(inputs["gamma"], np.float32)
    beta = np.asarray(inputs["beta"], np.float32)
    return {
        "Wl16": Wl.astype(ml_dtypes.bfloat16),
        "Wr16": Wr.astype(ml_dtypes.bfloat16),
        "attb16": att.reshape(L, 1, H * C).astype(ml_dtypes.bfloat16),
        "bc": (bl + br).reshape(L, 1, D),
        "cvec": (bl + gat_bias).reshape(L, 1, D),
        "gamma": gamma.reshape(L, 1, D),
        "beta": beta.reshape(L, 1, D),
        "ident": np.eye(P, dtype=np.float32),
    }


def make_in_maps(inputs, pre, cfg):
    x = np.asarray(inputs["fine_poi_x"], np.float32)
    shared = make_host_inputs(inputs, cfg)
    in_maps = []
    for c in range(cfg.M):
        m = dict(shared)
        m["x_shard"] = np.ascontiguousarray(x[c * cfg.shard : (c + 1) * cfg.shard])
        for k in ("src_idx", "S", "ST"):
            m[k] = pre[c][k]
        in_maps.append(m)
    return in_maps


# ----------------------------------------------------------------------------
# program assembly + execution
# ----------------------------------------------------------------------------

_CACHE = {}


def _build_program(cfg, meta):
    key = (cfg.N, cfg.D, cfg.H, cfg.L, cfg.M, tuple(meta["K_list"]))
    if key in _CACHE:
        return _CACHE[key]
    nc = bacc.Bacc(
        "TRN2", target_bir_lowering=False, debug=False, num_devices=cfg.M
    )
    TOTK = meta["TOTK"]
    io = {}
    io["x_shard"] = nc.dram_tensor(
        "x_shard", [cfg.shard, cfg.D], F32, kind="ExternalInput"
    ).ap()
    io["src_idx"] = nc.dram_tensor(
        "src_idx", [P, TOTK], I32, kind="ExternalInput"
    ).ap()
    io["S"] = nc.dram_tensor("S", [P, TOTK * P], BF16, kind="ExternalInput").ap()
    io["ST"] = nc.dram_tensor("ST", [P, TOTK * P], BF16, kind="ExternalInput").ap()
    io["Wl16"] = nc.dram_tensor(
        "Wl16", [cfg.L, cfg.D, cfg.D], BF16, kind="ExternalInput"
    ).ap()
    io["Wr16"] = nc.dram_tensor(
        "Wr16", [cfg.L, cfg.D, cfg.D], BF16, kind="ExternalInput"
    ).ap()
    io["attb16"] = nc.dram_tensor(
        "attb16", [cfg.L, 1, cfg.D], BF16, kind="ExternalInput"
    ).ap()
    for nm in ["bc", "cvec", "gamma", "beta"]:
        io[nm] = nc.dram_tensor(
            nm, [cfg.L, 1, cfg.D], F32, kind="ExternalInput"
        ).ap()
    io["ident"] = nc.dram_tensor("ident", [P, P], F32, kind="ExternalInput").ap()
    io["y"] = nc.dram_tensor(
        "y", [cfg.shard, cfg.D], F32, kind="ExternalOutput"
    ).ap()

    with tile.TileContext(nc) as tc:
        build(tc, io, cfg, meta)
    nc.compile()
    _CACHE[key] = nc
    return nc


def kernel(**inputs):
    from concourse import bass_utils

    cfg = Cfg()
    pre, meta = preprocess(inputs["edge_index"], cfg)
    nc = _build_program(cfg, meta)
    in_maps = make_in_maps(inputs, pre, cfg)
    res = bass_utils.run_bass_kernel_spmd(nc, in_maps, core_ids=list(range(cfg.M)))
    out = np.concatenate([res.results[c]["y"] for c in range(cfg.M)], axis=0)
    return out.astype(np.float32)


# revision 7
# speedup vs baseline: 1.2838x; 1.0226x over previous
"""GATv2 (3-layer, 4-head) message-passing kernel for Trainium2, 8-core SPMD.

Strategy (per sharding hint): nodes sharded contiguously across 8 cores;
edges partitioned by destination; per-layer AllGather of the source-side
transform xl = x @ Wl (bf16) so each core can gather arbitrary source rows;
segment softmax / scatter-add stay local per destination shard.

Algebra: with m_e = xl[src_e] + xr[dst_e] (xr = x@Wr + (bl+br)) and
ee = exp(att . leaky(m)), the attention output per destination is
    sum_e ee_e * m_e / denom - xr[dst] + (bl + gat_bias)
so only one row-gather per edge (xl) is needed; the xr side is a
PE segment-broadcast (ST_k^T @ xr_chunk).

v2 changes vs the 5.6ms baseline (HW-measured):
- selection matrices S (scatter) and ST (broadcast) are static per edge
  structure -> precomputed on host, streamed from DRAM on the scalar/sync
  hwdge queues (no on-chip is_equal builds, no gpsimd broadcast DMAs).
- per-chunk K is variable (max over cores) -> ~6% fewer indirect gathers;
  the indirect gather is the hard bottleneck at ~1.05us/instruction fixed.
- all K ST-matmuls accumulate into one 5-bank PSUM region; a single DVE
  add (+ gathered xl) builds m (replaces 19 tiny adds).
- leaky relu is one scalar-engine Prelu (AP alpha); rstd = exp(-0.5*ln(v))
  keeps every scalar activation in one table set (no ACT_TABLE thrash).
- x (residual, f32), xT (bf16 lhsT), xr (bf16) are SBUF-resident.
"""

import os
import sys

sys.path.insert(0, "/opt/trn_rl_repo")

import ml_dtypes
import numpy as np

import concourse.bass as bass
import concourse.bacc as bacc
import concourse.tile as tile
from concourse import mybir
from concourse.bass import IndirectOffsetOnAxis

F32 = mybir.dt.float32
BF16 = mybir.dt.bfloat16
I32 = mybir.dt.int32
AF = mybir.ActivationFunctionType
ALU = mybir.AluOpType
AX = mybir.AxisListType

P = 128
NEG_SLOPE = 0.2
LN_EPS = 1e-5
DENOM_EPS = 1e-30
DBG_LAYERS = int(os.environ.get("GAT_LAYERS", "0"))  # 0 = all


class Cfg:
    def __init__(self, N=50000, D=128, H=4, L=3, n_cores=8):
        self.N, self.D, self.H, self.L, self.M = N, D, H, L, n_cores
        self.C = D // H
        assert N % n_cores == 0
        self.shard = N // n_cores
        self.chunks = (self.shard + P - 1) // P


# ----------------------------------------------------------------------------
# Host preprocessing: append self loops, sort by dst, pack per-core chunk
# edge lists, build S / ST selection matrices.
# ----------------------------------------------------------------------------

def preprocess(edge_index, cfg):
    N, M, shard, chunks = cfg.N, cfg.M, cfg.shard, cfg.chunks
    ei = np.asarray(edge_index)
    loops = np.arange(N, dtype=np.int64)
    src = np.concatenate([ei[0].astype(np.int64), loops])
    dst = np.concatenate([ei[1].astype(np.int64), loops])
    order = np.argsort(dst, kind="stable")
    src_s, dst_s = src[order], dst[order]

    # per-(core, chunk) edge lists
    per_core = []
    cnts = np.zeros((M, chunks), dtype=np.int64)
    for c in range(M):
        lo, hi = np.searchsorted(dst_s, [c * shard, (c + 1) * shard])
        d_loc = dst_s[lo:hi] - c * shard
        s_loc = src_s[lo:hi]
        ch = d_loc // P
        chunk_edges = []
        for t in range(chunks):
            m = ch == t
            chunk_edges.append((s_loc[m], (d_loc[m] - t * P)))
            cnts[c, t] = int(m.sum())
        per_core.append(chunk_edges)

    K_list = [int(-(-cnts[:, t].max() // P)) for t in range(chunks)]
    offs = np.concatenate([[0], np.cumsum(K_list)]).astype(np.int64)
    TOTK = int(offs[-1])

    pre = []
    for c in range(M):
        src_idx = np.zeros((P, TOTK), dtype=np.int32)
        S = np.zeros((P, TOTK * P), dtype=ml_dtypes.bfloat16)
        ST = np.zeros((P, TOTK * P), dtype=ml_dtypes.bfloat16)
        for t in range(chunks):
            sl, dl = per_core[c][t]
            j = np.arange(len(sl))
            p, k = j % P, j // P
            src_idx[p, offs[t] + k] = sl
            # S[p_slot, (off+k)*P + d] = 1 ; ST[d, (off+k)*P + p_slot] = 1
            S[p, (offs[t] + k) * P + dl] = 1
            ST[dl, (offs[t] + k) * P + p] = 1
        pre.append({"src_idx": src_idx, "S": S, "ST": ST})
    meta = {"K_list": K_list, "offs": offs, "TOTK": TOTK}
    return pre, meta


# ----------------------------------------------------------------------------
# Kernel builder
# ----------------------------------------------------------------------------

def build(tc, io, cfg, meta):
    from contextlib import ExitStack

    nc = tc.nc
    D, H, L, C = cfg.D, cfg.H, cfg.L, cfg.C
    shard, chunks = cfg.shard, cfg.chunks
    K_list, offs = meta["K_list"], meta["offs"]

    ctx = ExitStack()
    dram = ctx.enter_context(tc.tile_pool(name="drampool", bufs=1, space="DRAM"))
    consts = ctx.enter_context(tc.tile_pool(name="consts", bufs=1))
    state = ctx.enter_context(tc.tile_pool(name="state", bufs=1))
    lconsts = ctx.enter_context(tc.tile_pool(name="lconsts", bufs=2))
    nodep = ctx.enter_context(tc.tile_pool(name="nodep", bufs=3))
    idxp = ctx.enter_context(tc.tile_pool(name="idxp", bufs=3))
    edgep = ctx.enter_context(tc.tile_pool(name="edgep", bufs=3))
    smallp = ctx.enter_context(tc.tile_pool(name="smallp", bufs=3))
    ps_big = ctx.enter_context(tc.tile_pool(name="ps_big", bufs=1, space="PSUM"))
    ps_o = ctx.enter_context(tc.tile_pool(name="ps_o", bufs=1, space="PSUM"))
    ps_n = ctx.enter_context(tc.tile_pool(name="ps_n", bufs=1, space="PSUM"))
    ps_t = ctx.enter_context(tc.tile_pool(name="ps_t", bufs=1, space="PSUM"))

    KMAX = max(K_list)

    # internal DRAM
    xl_sh = [dram.tile([shard, D], BF16, name=f"xl_sh{l}") for l in range(L)]
    xl_all = [
        dram.tile([cfg.N, D], BF16, name=f"xl_all{l}", addr_space="Shared")
        for l in range(L)
    ]

    # SBUF-resident constants / state
    ident_sb = consts.tile([P, P], F32, name="ident_sb")
    nc.sync.dma_start(out=ident_sb[:], in_=io["ident"][:, :])
    alpha_sb = consts.tile([P, 1], F32, name="alpha_sb")
    nc.vector.memset(alpha_sb[:, :], NEG_SLOPE)
    x_sb = state.tile([P, chunks, D], F32, name="x_sb")
    xT_sb = state.tile([P, chunks * P], BF16, name="xT_sb")
    xr_sb = state.tile([P, chunks, D], BF16, name="xr_sb")
    nc.vector.memset(xr_sb[:, :, :], 0.0)

    def node_step(l, t, wl, wr, bc):
        """xl = x@Wl -> xl_sh[l]; xr = x@Wr + bc -> xr_sb (bf16)."""
        nt = min(P, shard - t * P)
        lhsT = xT_sb[:, t * P : t * P + nt]
        ps_xl = ps_n.tile([P, D], F32, name="ps_xl", tag="ps_n")
        nc.tensor.matmul(
            out=ps_xl[:nt, :], lhsT=lhsT, rhs=wl[:], start=True, stop=True
        )
        xl_o = nodep.tile([P, D], BF16, name="xl_o")
        nc.scalar.activation(out=xl_o[:nt, :], in_=ps_xl[:nt, :], func=AF.Copy)
        nc.sync.dma_start(out=xl_sh[l][t * P : t * P + nt, :], in_=xl_o[:nt, :])
        ps_xr = ps_n.tile([P, D], F32, name="ps_xr", tag="ps_n")
        nc.tensor.matmul(
            out=ps_xr[:nt, :], lhsT=lhsT, rhs=wr[:], start=True, stop=True
        )
        nc.vector.tensor_tensor(
            out=xr_sb[:nt, t, :], in0=ps_xr[:nt, :], in1=bc[:nt, :], op=ALU.add
        )

    def allgather(l):
        ins, outs = xl_sh[l][:, :], xl_all[l][:, :]
        nc.gpsimd.collective_compute(
            "AllGather",
            ALU.bypass,
            replica_groups=[list(range(cfg.M))],
            ins=[ins.opt()],
            outs=[outs.opt()],
        )

    def load_node_consts(l):
        wl = lconsts.tile([P, D], BF16, name="wl_sb")
        nc.sync.dma_start(out=wl[:], in_=io["Wl16"][l, :, :])
        wr = lconsts.tile([P, D], BF16, name="wr_sb")
        nc.sync.dma_start(out=wr[:], in_=io["Wr16"][l, :, :])
        bc = lconsts.tile([P, D], F32, name="bc_sb")
        nc.gpsimd.dma_start(out=bc[:], in_=_row_bcast(io["bc"], l, P, D))
        return wl, wr, bc

    # ------------------------------------------------------------------
    # prologue: x -> x_sb, xT; node(0) interleaved; AllGather(0) in halves
    # ------------------------------------------------------------------
    for t in range(chunks):
        nt = min(P, shard - t * P)
        nc.sync.dma_start(
            out=x_sb[:nt, t, :], in_=io["x_shard"][t * P : t * P + nt, :]
        )
    wl0, wr0, bc0 = load_node_consts(0)
    for t in range(chunks):
        nt = min(P, shard - t * P)
        psT = ps_t.tile([P, P], F32, name="psT", tag="psT")
        nc.tensor.transpose(
            out=psT[:, :nt], in_=x_sb[:nt, t, :], identity=ident_sb[:nt, :nt]
        )
        nc.scalar.activation(
            out=xT_sb[:, t * P : t * P + nt], in_=psT[:, :nt], func=AF.Copy
        )
        node_step(0, t, wl0, wr0, bc0)
    allgather(0)

    L_eff = DBG_LAYERS if DBG_LAYERS else L
    nconsts = (wl0, wr0, bc0)
    for l in range(L_eff):
        # tail constants for this layer; node constants for the next
        attb_sb = lconsts.tile([P, D], BF16, name="attb_sb")
        nc.gpsimd.dma_start(out=attb_sb[:], in_=_row_bcast(io["attb16"], l, P, D))
        cvec_sb = lconsts.tile([P, D], F32, name="cvec_sb")
        nc.gpsimd.dma_start(out=cvec_sb[:], in_=_row_bcast(io["cvec"], l, P, D))
        gamma_sb = lconsts.tile([P, D], F32, name="gamma_sb")
        nc.gpsimd.dma_start(out=gamma_sb[:], in_=_row_bcast(io["gamma"], l, P, D))
        beta_sb = lconsts.tile([P, D], F32, name="beta_sb")
        nc.gpsimd.dma_start(out=beta_sb[:], in_=_row_bcast(io["beta"], l, P, D))
        if l < L_eff - 1:
            nconsts = load_node_consts(l + 1)

        # --------------------------------------------------------------
        # edge phase (node phase of layer l+1 interleaved per chunk)
        # --------------------------------------------------------------
        for ch in range(chunks):
            nt = min(P, shard - ch * P)
            K = K_list[ch]
            off = int(offs[ch])

            idx_sb = idxp.tile([P, KMAX], I32, name="idx_sb")
            nc.sync.dma_start(out=idx_sb[:, :K], in_=io["src_idx"][:, off : off + K])
            S_sb = edgep.tile([P, KMAX * P], BF16, name="S_sb")
            nc.scalar.dma_start(
                out=S_sb[:, : K * P], in_=io["S"][:, off * P : (off + K) * P]
            )
            ST_sb = edgep.tile([P, KMAX * P], BF16, name="ST_sb")
            nc.sync.dma_start(
                out=ST_sb[:, : K * P], in_=io["ST"][:, off * P : (off + K) * P]
            )

            # gather xl rows (the bottleneck: one indirect DMA per k-tile)
            g2 = edgep.tile([P, KMAX, D], BF16, name="g2")
            for k in range(K):
                nc.gpsimd.indirect_dma_start(
                    out=g2[:, k, :],
                    out_offset=None,
                    in_=xl_all[l][:, :],
                    in_offset=IndirectOffsetOnAxis(ap=idx_sb[:, k : k + 1], axis=0),
                )

            # m = (ST_k^T @ xr_chunk) + g2
            pm = ps_big.tile([P, KMAX * D], F32, name="pm", tag="pm")
            for k in range(K):
                nc.tensor.matmul(
                    out=pm[:, k * D : (k + 1) * D],
                    lhsT=ST_sb[:, k * P : (k + 1) * P],
                    rhs=xr_sb[:, ch, :],
                    start=True,
                    stop=True,
                )
            m_t = edgep.tile([P, KMAX, D], BF16, name="m_t")
            nc.vector.tensor_tensor(
                out=m_t[:, :K, :].rearrange("p k d -> p (k d)"),
                in0=pm[:, : K * D],
                in1=g2[:, :K, :].rearrange("p k d -> p (k d)"),
                op=ALU.add,
            )

            # leaky relu on the scalar engine (Prelu with AP alpha)
            lk = edgep.tile([P, KMAX, D], BF16, name="lk")
            nc.scalar.activation(
                out=lk[:, :K, :].rearrange("p k d -> p (k d)"),
                in_=m_t[:, :K, :].rearrange("p k d -> p (k d)"),
                func=AF.Prelu,
                alpha=alpha_sb[:, 0:1],
            )

            # attention logits and exp
            tt = edgep.tile([P, KMAX, D], BF16, name="tt")
            nc.vector.tensor_tensor(
                out=tt[:, :K, :],
                in0=lk[:, :K, :],
                in1=attb_sb[:, :].unsqueeze(1).to_broadcast([P, K, D]),
                op=ALU.mult,
            )
            lg = smallp.tile([P, KMAX, H], F32, name="lg")
            nc.vector.reduce_sum(
                out=lg[:, :K, :],
                in_=tt[:, :K, :].rearrange("p k (h c) -> p k h c", h=H),
                axis=AX.X,
            )
            zee = edgep.tile([P, KMAX, D + H], BF16, name="zee")
            nc.scalar.activation(
                out=zee[:, :K, D : D + H], in_=lg[:, :K, :], func=AF.Exp
            )
            nc.vector.tensor_tensor(
                out=zee[:, :K, 0:D].rearrange("p k (h c) -> p k h c", h=H),
                in0=m_t[:, :K, :].rearrange("p k (h c) -> p k h c", h=H),
                in1=zee[:, :K, D : D + H].unsqueeze(3).to_broadcast([P, K, H, C]),
                op=ALU.mult,
            )

            # segment sums on PE: po[d, 0:D] = sum ee*m ; po[d, D:D+H] = denom
            po = ps_o.tile([P, D + H], F32, name="po", tag="po")
            for k in range(K):
                nc.tensor.matmul(
                    out=po[:, :],
                    lhsT=S_sb[:, k * P : (k + 1) * P],
                    rhs=zee[:, k, :],
                    start=(k == 0),
                    stop=(k == K - 1),
                )

            # normalize, subtract xr, add cvec, residual, LN
            dn = smallp.tile([P, H], F32, name="dn")
            nc.vector.tensor_scalar(
                out=dn[:, :], in0=po[:, D : D + H], scalar1=DENOM_EPS,
                scalar2=None, op0=ALU.add,
            )
            rd = smallp.tile([P, H], F32, name="rd")
            nc.vector.reciprocal(out=rd[:, :], in_=dn[:, :])
            onrm = smallp.tile([P, D], F32, name="onrm")
            nc.vector.tensor_tensor(
                out=onrm[:, :].rearrange("p (h c) -> p h c", h=H),
                in0=po[:, 0:D].rearrange("p (h c) -> p h c", h=H),
                in1=rd[:, :].unsqueeze(2).to_broadcast([P, H, C]),
                op=ALU.mult,
            )
            t1 = smallp.tile([P, D], F32, name="t1")
            nc.vector.tensor_tensor(
                out=t1[:nt, :], in0=onrm[:nt, :], in1=xr_sb[:nt, ch, :],
                op=ALU.subtract,
            )
            t2 = smallp.tile([P, D], F32, name="t2")
            nc.vector.tensor_tensor(
                out=t2[:nt, :], in0=t1[:nt, :], in1=cvec_sb[:nt, :], op=ALU.add
            )
            t3 = smallp.tile([P, D], F32, name="t3")
            nc.vector.tensor_tensor(
                out=t3[:nt, :], in0=t2[:nt, :], in1=x_sb[:nt, ch, :], op=ALU.add
            )

            st6 = smallp.tile([P, 6], F32, name="st6")
            nc.vector.bn_stats(out=st6[:nt, :], in_=t3[:nt, :])
            mv = smallp.tile([P, 2], F32, name="mv")
            nc.vector.bn_aggr(out=mv[:nt, :], in_=st6[:nt, :])
            # rstd = rsqrt(var + eps) on DVE: quake seed + 2 Newton steps
            # (keeps the scalar engine on a single act table set)
            veps = smallp.tile([P, 1], F32, name="veps")
            nc.vector.tensor_scalar(
                out=veps[:nt, :], in0=mv[:nt, 1:2], scalar1=LN_EPS, scalar2=None,
                op0=ALU.add,
            )
            seedi = smallp.tile([P, 1], I32, name="seedi")
            nc.vector.tensor_scalar(
                out=seedi[:nt, :], in0=veps[:nt, :].bitcast(I32), scalar1=1,
                scalar2=None, op0=ALU.logical_shift_right,
            )
            seedf = smallp.tile([P, 1], I32, name="seedf")
            nc.vector.tensor_scalar(
                out=seedf[:nt, :], in0=seedi[:nt, :], scalar1=0x5F3759DF,
                scalar2=-1, op0=ALU.subtract, op1=ALU.mult,
            )
            hhalf = smallp.tile([P, 1], F32, name="hhalf")
            nc.vector.tensor_scalar(
                out=hhalf[:nt, :], in0=veps[:nt, :], scalar1=0.5, scalar2=None,
                op0=ALU.mult,
            )
            y0 = seedf[:nt, :].bitcast(F32)
            nta = smallp.tile([P, 1], F32, name="nta")
            nc.vector.tensor_tensor(out=nta[:nt, :], in0=y0, in1=y0, op=ALU.mult)
            ntb = smallp.tile([P, 1], F32, name="ntb")
            nc.vector.tensor_scalar(
                out=ntb[:nt, :], in0=nta[:nt, :], scalar1=hhalf[:nt, :],
                scalar2=1.5, op0=ALU.mult, op1=ALU.subtract,
            )
            y1n = smallp.tile([P, 1], F32, name="y1n")
            nc.vector.tensor_tensor(out=y1n[:nt, :], in0=ntb[:nt, :], in1=y0, op=ALU.mult)
            nc.vector.tensor_tensor(
                out=nta[:nt, :], in0=y1n[:nt, :], in1=y1n[:nt, :], op=ALU.mult
            )
            nc.vector.tensor_scalar(
                out=ntb[:nt, :], in0=nta[:nt, :], scalar1=hhalf[:nt, :],
                scalar2=1.5, op0=ALU.mult, op1=ALU.subtract,
            )
            rstd = smallp.tile([P, 1], F32, name="rstd")
            nc.vector.tensor_tensor(
                out=rstd[:nt, :], in0=ntb[:nt, :], in1=y1n[:nt, :], op=ALU.mult
            )

            y1 = smallp.tile([P, D], F32, name="y1")
            nc.vector.tensor_scalar(
                out=y1[:nt, :], in0=t3[:nt, :], scalar1=mv[:nt, 0:1],
                scalar2=rstd[:nt, :], op0=ALU.subtract, op1=ALU.mult,
            )
            y2 = smallp.tile([P, D], F32, name="y2")
            nc.vector.tensor_tensor(
                out=y2[:nt, :], in0=y1[:nt, :], in1=gamma_sb[:nt, :], op=ALU.mult
            )
            y3 = smallp.tile([P, D], F32, name="y3")
            nc.vector.tensor_tensor(
                out=y3[:nt, :], in0=y2[:nt, :], in1=beta_sb[:nt, :], op=ALU.add
            )

            if l < L_eff - 1:
                nc.scalar.activation(
                    out=x_sb[:nt, ch, :], in_=y3[:nt, :], func=AF.Relu
                )
                psT2 = ps_t.tile([P, P], F32, name="psT2", tag="psT")
                nc.tensor.transpose(
                    out=psT2[:, :nt], in_=x_sb[:nt, ch, :],
                    identity=ident_sb[:nt, :nt],
                )
                nc.scalar.activation(
                    out=xT_sb[:, ch * P : ch * P + nt], in_=psT2[:, :nt],
                    func=AF.Copy,
                )
                node_step(l + 1, ch, *nconsts)
            else:
                nc.sync.dma_start(
                    out=io["y"][ch * P : ch * P + nt, :], in_=y3[:nt, :]
                )
        if l < L_eff - 1:
            allgather(l + 1)

    ctx.close()


def _row_bcast(ap, row, parts, d):
    """AP reading row `row` of a [R, 1, D] DRAM tensor, replicated across
    `parts` partitions (partition step 0)."""
    flat = ap[row] if ap.ndim == 3 else ap[row : row + 1]
    base = flat.opt()
    return bass.AP(tensor=base.tensor, offset=row * d, ap=[[0, parts], [1, d]])


# ----------------------------------------------------------------------------
# host-side inputs
# ----------------------------------------------------------------------------

def make_host_inputs(inputs, cfg):
    L, D, H, C = cfg.L, cfg.D, cfg.H, cfg.C
    Wl = np.asarray(inputs["Wl"], np.float32)
    Wr = np.asarray(inputs["Wr"], np.float32)
    bl = np.asarray(inputs["bl"], np.float32)
    br = np.asarray(inputs["br"], np.float32)
    att = np.asarray(inputs["att"], np.float32)
    gat_bias = np.asarray(inputs["bias"], np.float32)
    gamma = np.asarray(inputs["gamma"], np.float32)
    beta = np.asarray(inputs["beta"], np.float32)
    return {
        "Wl16": Wl.astype(ml_dtypes.bfloat16),
        "Wr16": Wr.astype(ml_dtypes.bfloat16),
        "attb16": att.reshape(L, 1, H * C).astype(ml_dtypes.bfloat16),
        "bc": (bl + br).reshape(L, 1, D),
        "cvec": (bl + gat_bias).reshape(L, 1, D),
        "gamma": gamma.reshape(L, 1, D),
        "beta": beta.reshape(L, 1, D),
        "ident": np.eye(P, dtype=np.float32),
    }


def make_in_maps(inputs, pre, cfg):
    x = np.asarray(inputs["fine_poi_x"], np.float32)
    shared = make_host_inputs(inputs, cfg)
    in_maps = []
    for c in range(cfg.M):
        m = dict(shared)
        m["x_shard"] = np.ascontiguousarray(x[c * cfg.shard : (c + 1) * cfg.shard])
        for k in ("src_idx", "S", "ST"):
            m[k] = pre[c][k]
        in_maps.append(m)
    return in_maps


# ----------------------------------------------------------------------------
# program assembly + execution
# ----------------------------------------------------------------------------

_CACHE = {}


def _build_program(cfg, meta):
    key = (cfg.N, cfg.D, cfg.H, cfg.L, cfg.M, tuple(meta["K_list"]))
    if key in _CACHE:
        return _CACHE[key]
    nc = bacc.Bacc(
        "TRN2", target_bir_lowering=False, debug=False, num_devices=cfg.M
    )
    TOTK = meta["TOTK"]
    io = {}
    io["x_shard"] = nc.dram_tensor(
        "x_shard", [cfg.shard, cfg.D], F32, kind="ExternalInput"
    ).ap()
    io["src_idx"] = nc.dram_tensor(
        "src_idx", [P, TOTK], I32, kind="ExternalInput"
    ).ap()
    io["S"] = nc.dram_tensor("S", [P, TOTK * P], BF16, kind="ExternalInput").ap()
    io["ST"] = nc.dram_tensor("ST", [P, TOTK * P], BF16, kind="ExternalInput").ap()
    io["Wl16"] = nc.dram_tensor(
        "Wl16", [cfg.L, cfg.D, cfg.D], BF16, kind="ExternalInput"
    ).ap()
    io["Wr16"] = nc.dram_tensor(
        "Wr16", [cfg.L, cfg.D, cfg.D], BF16, kind="ExternalInput"
    ).ap()
    io["attb16"] = nc.dram_tensor(
        "attb16", [cfg.L, 1, cfg.D], BF16, kind="ExternalInput"
    ).ap()
    for nm in ["bc", "cvec", "gamma", "beta"]:
        io[nm] = nc.dram_tensor(
            nm, [cfg.L, 1, cfg.D], F32, kind="ExternalInput"
        ).ap()
    io["ident"] = nc.dram_tensor("ident", [P, P], F32, kind="ExternalInput").ap()
    io["y"] = nc.dram_tensor(
        "y", [cfg.shard, cfg.D], F32, kind="ExternalOutput"
    ).ap()

    with tile.TileContext(nc) as tc:
        build(tc, io, cfg, meta)
    nc.compile()
    _CACHE[key] = nc
    return nc


def kernel(**inputs):
    from concourse import bass_utils

    cfg = Cfg()
    pre, meta = preprocess(inputs["edge_index"], cfg)
    nc = _build_program(cfg, meta)
    in_maps = make_in_maps(inputs, pre, cfg)
    res = bass_utils.run_bass_kernel_spmd(nc, in_maps, core_ids=list(range(cfg.M)))
    out = np.concatenate([res.results[c]["y"] for c in range(cfg.M)], axis=0)
    return out.astype(np.float32)
